# revision 2
# baseline (speedup 1.0000x reference)
"""Multi-head differential attention (full-width) on 8 Trainium2 NeuronCores.

Contract: kernel(**inputs) takes the FULL inputs of reference.setup_inputs()
and returns the FULL [8, 512, 8192] float32 output.

Strategy: pure data-parallel over batch — one batch element per NeuronCore.

Key algebraic optimization: softmax(q_h k_hT) = softmax(x M_h xT) with
M_h = Wq_hT @ Wk_h precomputed HOST-side (a pure weight transformation, like
the lam precompute).  This removes the k-projection entirely and folds the
q-projection into a single xM GEMM: the score path drops from
2*(S*D*D) + S*S*D MACs to S*D*D + S*S*D per half — ~47% less tensor-engine
work in phase A.  Biases fold exactly: row-constant terms are softmax
invariant; the only surviving term is 1·(x · Wk_hT bq_h)ᵀ, a per-j scalar
added during the PSUM→SBUF copy of t = xM.

Per core, a single fused Bass/Tile kernel computes:
  - tT = (x M_h)T j-block by j-block (f32r matmuls at full PE rate), with
    the score matmuls fused into the same loop (rotating 2-tile window),
  - softmax via ScalarE Exp with accum_out row-sums (scores bounded ~±25,
    no max-subtraction needed in fp32),
  - differential combine P = e1/d1 - lam ⊙ e2/d2 on VectorE,
  - PE-transpose of P, then a fused v-projection + attention-V matmul loop
    that streams Wv (bf16) and writes each 512-column output tile as it
    completes.

Weights are repacked host-side so every DMA lands with long contiguous
per-partition runs.
"""
import ml_dtypes
import numpy as np
from contextlib import ExitStack

import concourse.bass as bass
import concourse.mybir as mybir
import concourse.tile as tile
from concourse.bass_utils import run_bass_kernel_spmd
from concourse.masks import make_identity

F32 = mybir.dt.float32
F32R = mybir.dt.float32r
BF16 = mybir.dt.bfloat16
P = 128
B = 8
S = 512          # sequence length (= d_head for the lambda broadcast)
DM = 4096        # model dim (xM contraction)
DH = 4096        # width of each q/k half (score contraction)
D2 = 2 * DH      # v-projection output width
NQB = S // P     # 4 query blocks
NT = DM // P     # 32 contraction tiles
HOB = DH // P    # 32 j-blocks per half
NODT = D2 // 512  # 16 output column tiles
WV_CH = 8        # dq-tiles per streamed Wv chunk
NCH = NT // WV_CH
SCALE = float(1.0 / np.sqrt(512.0))

MAX_WAITS = 1  # this walrus build allows one sync-wait per instruction


def _split_sync_waits(nc):
    """Move excess per-instruction sync waits onto preceding no-ops (same
    engine, program order) — semantically identical, keeps walrus happy."""
    for f in nc.m.functions:
        for bb in f.blocks:
            new_insts = []
            for inst in bb.instructions:
                si = inst.sync_info
                if si is not None and si.on_wait and len(si.on_wait) > MAX_WAITS:
                    waits = list(si.on_wait)
                    excess, keep = waits[MAX_WAITS:], waits[:MAX_WAITS]
                    for ci in range(0, len(excess), MAX_WAITS):
                        new_insts.append(mybir.InstNoOp(
                            name=f"{inst.name}-waitsplit{ci}",
                            engine=inst.engine, ins=[], outs=[],
                            sync_info=mybir.SyncInfo(
                                on_wait=excess[ci:ci + MAX_WAITS], on_update=[]),
                            text_hint="waitsplit"))
                    si.on_wait = keep
                new_insts.append(inst)
            bb.instructions = new_insts


def build_nc():
    nc = bass.Bass()
    xT = nc.declare_dram_parameter("xT", [P, NT, S], F32R, isOutput=False)
    xTb = nc.declare_dram_parameter("xTb", [P, NT, S], BF16, isOutput=False)
    m = nc.declare_dram_parameter("m", [2, HOB, P, NT, P], F32R, isOutput=False)
    wv = nc.declare_dram_parameter("wv", [NODT, P, NT, 512], BF16, isOutput=False)
    wtld = nc.declare_dram_parameter("wtld", [P, 2 * HOB], F32, isOutput=False)
    vb = nc.declare_dram_parameter("vb", [D2], BF16, isOutput=False)
    lam = nc.declare_dram_parameter("lam", [S], F32R, isOutput=False)
    ones = nc.declare_dram_parameter("ones", [P], F32R, isOutput=False)
    onesb = nc.declare_dram_parameter("onesb", [P], BF16, isOutput=False)
    out = nc.declare_dram_parameter("out", [S, D2], F32, isOutput=True)

    with tile.TileContext(nc) as tc, ExitStack() as top:
        const = top.enter_context(tc.tile_pool(name="const", bufs=1))
        xT_sb = const.tile([P, NT, S], F32R, name="xT_sb")
        for xc in range(4):
            nc.sync.dma_start(xT_sb[:, xc * (NT // 4):(xc + 1) * (NT // 4), :],
                              xT[:, xc * (NT // 4):(xc + 1) * (NT // 4), :])
        wtld_sb = const.tile([P, 2 * HOB], F32, name="wtld_sb")
        nc.sync.dma_start(wtld_sb[:], wtld[:])
        lam_sb = const.tile([1, S], F32R, name="lam_sb")
        nc.sync.dma_start(lam_sb[:], lam[None, :])
        ones_row = const.tile([1, P], F32R, name="ones_row")
        nc.sync.dma_start(ones_row[:], ones[None, :])
        onesb_row = const.tile([1, P], BF16, name="onesb_row")
        nc.sync.dma_start(onesb_row[:], onesb[None, :])
        xTb_sb = const.tile([P, NT, S], BF16, name="xTb_sb")
        wv0 = const.tile([P, WV_CH, 512], BF16, name="wv0")
        vbt0 = const.tile([1, 512], BF16, name="vbt0")
        ident = const.tile([P, P], F32, name="ident")
        make_identity(nc, ident[:])

        # lam broadcast to all 128 partitions via K=1 matmul
        lam_bc = const.tile([P, S], F32, name="lam_bc")
        with tc.tile_pool(name="ps_misc", bufs=1, space="PSUM") as ps_misc:
            pt = ps_misc.tile([P, 512], F32, name="lam_ps")
            nc.tensor.matmul(pt[:], ones_row[:], lam_sb[:], start=True, stop=True)
            nc.vector.tensor_copy(out=lam_bc[:], in_=pt[:])

        e_sb = const.tile([P, 2, NQB, S], F32, name="e_sb")
        d_sb = const.tile([P, 2, NQB], F32, name="d_sb")
        r_sb = const.tile([P, 2, NQB], F32, name="r_sb")
        P_sb = const.tile([P, NQB, S], F32, name="P_sb")
        PT_sb = const.tile([P, S // P, S], BF16, name="PT_sb")

        # ---- Phase A: fused xM projection + score accumulation ----
        with ExitStack() as phA:
            mw = phA.enter_context(tc.tile_pool(name="mw", bufs=4))
            tsb = phA.enter_context(tc.tile_pool(name="tsb", bufs=6))
            ps_t = phA.enter_context(tc.tile_pool(name="ps_t", bufs=3, space="PSUM"))
            ps_scores = phA.enter_context(tc.tile_pool(name="ps_scores", bufs=5, space="PSUM"))

            for h in range(2):
                sc_tiles = [ps_scores.tile([P, S], F32, name=f"sc_{h}_{qbk}", tag="sc")
                            for qbk in range(NQB)]
                prev_t = None

                def emit_scores(jb, t_sb):
                    for qbk in range(NQB):
                        nc.tensor.matmul(sc_tiles[qbk][:],
                                         t_sb[:, qbk * P:(qbk + 1) * P],
                                         xT_sb[:, jb, :],
                                         start=(jb == 0), stop=(jb == HOB - 1))

                for jb in range(HOB):
                    if h == 1 and jb == HOB - 8:
                        nc.sync.dma_start(xTb_sb[:], xTb[:])
                    if h == 1 and jb == HOB - 4:
                        nc.sync.dma_start(wv0[:], wv[0][:, 0:WV_CH, :])
                        nc.sync.dma_start(vbt0[:], vb[None, 0:512])
                    pt_ = ps_t.tile([P, S], F32, name="pt", tag="pp")
                    for cw in range(2):
                        mt = mw.tile([P, NT // 2, P], F32R, name="mt", tag="m")
                        nc.sync.dma_start(mt[:], m[h, jb][:, cw * (NT // 2):(cw + 1) * (NT // 2), :])
                        for tt in range(NT // 2):
                            t = cw * (NT // 2) + tt
                            nc.tensor.matmul(pt_[:], mt[:, tt, :], xT_sb[:, t, :],
                                             start=(t == 0), stop=(t == NT - 1))
                    t_sb = tsb.tile([P, S], F32R, name="t_sb", tag="t")
                    nc.vector.tensor_scalar(t_sb[:], pt_[:],
                                            wtld_sb[:, h * HOB + jb:h * HOB + jb + 1],
                                            None, mybir.AluOpType.add)
                    if prev_t is not None:
                        emit_scores(jb - 1, prev_t)
                    prev_t = t_sb
                emit_scores(HOB - 1, prev_t)
                for qbk in range(NQB):
                    nc.scalar.activation(e_sb[:, h, qbk, :], sc_tiles[qbk][:],
                                         mybir.ActivationFunctionType.Exp,
                                         accum_out=d_sb[:, h, qbk:qbk + 1])

        # ---- Phase B+C: combine + transpose overlapped with v projection/AV ----
        nc.vector.reciprocal(r_sb[:, :, :], d_sb[:, :, :])
        with ExitStack() as phC:
            cmb = phC.enter_context(tc.tile_pool(name="cmb", bufs=2))
            wvp = phC.enter_context(tc.tile_pool(name="wvp", bufs=3))
            vbp = phC.enter_context(tc.tile_pool(name="vbp", bufs=2))
            vsb = phC.enter_context(tc.tile_pool(name="vsb", bufs=2))
            osb = phC.enter_context(tc.tile_pool(name="osb", bufs=4))
            ps_tr = phC.enter_context(tc.tile_pool(name="ps_tr", bufs=2, space="PSUM"))
            ps_vp = phC.enter_context(tc.tile_pool(name="ps_vp", bufs=4, space="PSUM"))
            ps_av = phC.enter_context(tc.tile_pool(name="ps_av", bufs=2, space="PSUM"))

            def combine_and_transpose():
                for qbk in range(NQB):
                    tmp = cmb.tile([P, S], F32, name="tmp", tag="tmp")
                    nc.vector.tensor_tensor(tmp[:], e_sb[:, 1, qbk, :], lam_bc[:],
                                            mybir.AluOpType.mult)
                    nc.vector.tensor_scalar(tmp[:], tmp[:], r_sb[:, 1, qbk:qbk + 1], None,
                                            mybir.AluOpType.mult)
                    nc.vector.tensor_scalar(P_sb[:, qbk, :], e_sb[:, 0, qbk, :],
                                            r_sb[:, 0, qbk:qbk + 1], None,
                                            mybir.AluOpType.mult)
                    nc.vector.tensor_tensor(P_sb[:, qbk, :], P_sb[:, qbk, :], tmp[:],
                                            mybir.AluOpType.subtract)
                for qbk in range(NQB):
                    for kbk in range(S // P):
                        pt2 = ps_tr.tile([P, P], F32, name="pt2", tag="pt")
                        nc.tensor.transpose(pt2[:], P_sb[:, qbk, kbk * P:(kbk + 1) * P],
                                            ident[:])
                        nc.vector.tensor_copy(out=PT_sb[:, kbk, qbk * P:(qbk + 1) * P],
                                              in_=pt2[:])

            v_tiles = [None] * NODT

            def do_av(odt):
                for qbk in range(NQB):
                    pav = ps_av.tile([P, 512], F32, name="pav", tag="av")
                    for kbk in range(S // P):
                        nc.tensor.matmul(pav[:],
                                         PT_sb[:, kbk, qbk * P:(qbk + 1) * P],
                                         v_tiles[odt][:, kbk, :],
                                         start=(kbk == 0), stop=(kbk == S // P - 1))
                    o_st = osb.tile([P, 512], F32, name="o_st", tag="o")
                    nc.vector.tensor_copy(out=o_st[:], in_=pav[:])
                    nc.sync.dma_start(out[qbk * P:(qbk + 1) * P, odt * 512:(odt + 1) * 512],
                                      o_st[:])

            for odt in range(NODT):
                pv = [ps_vp.tile([P, 512], F32, name=f"pv{sb}", tag="vp")
                      for sb in range(NQB)]
                if odt == 0:
                    vbt = vbt0
                else:
                    vbt = vbp.tile([1, 512], BF16, name="vbt", tag="vb")
                    nc.sync.dma_start(vbt[:], vb[None, odt * 512:(odt + 1) * 512])
                for sb in range(NQB):
                    nc.tensor.matmul(pv[sb][:], onesb_row[:], vbt[:],
                                     start=True, stop=False)
                for c in range(NCH):
                    if odt == 0 and c == 0:
                        wvt = wv0
                    else:
                        wvt = wvp.tile([P, WV_CH, 512], BF16, name="wvt", tag="wv")
                        nc.sync.dma_start(wvt[:], wv[odt][:, c * WV_CH:(c + 1) * WV_CH, :])
                    for sb in range(NQB):
                        for tt in range(WV_CH):
                            t = c * WV_CH + tt
                            nc.tensor.matmul(pv[sb][:], xTb_sb[:, t, sb * P:(sb + 1) * P],
                                             wvt[:, tt, :],
                                             start=False, stop=(t == NT - 1))
                v_t = vsb.tile([P, S // P, 512], BF16, name="v_t", tag="v")
                for sb in range(NQB):
                    nc.vector.tensor_copy(out=v_t[:, sb, :], in_=pv[sb][:])
                v_tiles[odt] = v_t
                if odt == 0:
                    # PE chews on v-proj(0) while DVE does the combine and the
                    # transposes queue up behind it — hides the softmax tail.
                    combine_and_transpose()
                if odt >= 1:
                    do_av(odt - 1)
                    v_tiles[odt - 1] = None
            do_av(NODT - 1)

    _split_sync_waits(nc)
    return nc


def pack_shared(wq_w, wq_b, wk_w, wk_b, wv_w, wv_b,
                lambda_q1, lambda_k1, lambda_q2, lambda_k2):
    lam = (np.exp(lambda_q1 * lambda_k1) - np.exp(lambda_q2 * lambda_k2)
           + np.float32(0.8)).astype(np.float32)
    # M_h = Wq_h^T @ Wk_h  (scores_h = x M_h x^T); scale folded in.
    ms = []
    wt = []
    for h in range(2):
        wq_h = wq_w[h * DH:(h + 1) * DH]
        wk_h = wk_w[h * DH:(h + 1) * DH]
        M = (wq_h.T @ wk_h) * np.float32(SCALE)
        ms.append(M.reshape(NT, P, HOB, P).transpose(2, 1, 0, 3))
        # surviving bias term: 1·(x · Wk_h^T bq_h)^T, broadcast over q
        w_t = (wk_h.T @ wq_b[h * DH:(h + 1) * DH]) * np.float32(SCALE)
        wt.append(w_t.reshape(HOB, P).T)
    return {
        "m": np.ascontiguousarray(np.stack(ms)),
        "wtld": np.ascontiguousarray(np.concatenate(wt, axis=1)),
        "wv": np.ascontiguousarray(wv_w.reshape(NODT, 512, NT, P).transpose(0, 3, 2, 1)).astype(ml_dtypes.bfloat16),
        "vb": np.ascontiguousarray(wv_b).astype(ml_dtypes.bfloat16),
        "lam": lam,
        "ones": np.ones(P, np.float32),
        "onesb": np.ones(P, ml_dtypes.bfloat16),
    }


def make_in_maps(x, wq_w, wq_b, wk_w, wk_b, wv_w, wv_b,
                 lambda_q1, lambda_k1, lambda_q2, lambda_k2):
    shared = pack_shared(wq_w, wq_b, wk_w, wk_b, wv_w, wv_b,
                         lambda_q1, lambda_k1, lambda_q2, lambda_k2)
    maps = []
    for b in range(B):
        xp = np.ascontiguousarray(x[b].T.reshape(NT, P, S).transpose(1, 0, 2))
        maps.append({**shared, "xT": xp, "xTb": xp.astype(ml_dtypes.bfloat16)})
    return maps


_NC_CACHE = None


def get_nc():
    global _NC_CACHE
    if _NC_CACHE is None:
        _NC_CACHE = build_nc()
    return _NC_CACHE


def kernel(x, wq_w, wq_b, wk_w, wk_b, wv_w, wv_b,
           lambda_q1, lambda_k1, lambda_q2, lambda_k2):
    args = [np.asarray(a, dtype=np.float32) for a in
            (x, wq_w, wq_b, wk_w, wk_b, wv_w, wv_b,
             lambda_q1, lambda_k1, lambda_q2, lambda_k2)]
    nc = get_nc()
    in_maps = make_in_maps(*args)
    res = run_bass_kernel_spmd(nc, in_maps, list(range(B)))
    return np.stack([res.results[b]["out"] for b in range(B)]).astype(np.float32)


# revision 3
# speedup vs baseline: 1.0653x; 1.0653x over previous
"""Multi-head differential attention (full-width) on 8 Trainium2 NeuronCores.

Contract: kernel(**inputs) takes the FULL inputs of reference.setup_inputs()
and returns the FULL [8, 512, 8192] float32 output.

Strategy: pure data-parallel over batch — one batch element per NeuronCore.

Key algebraic optimization: softmax(q_h k_hT) = softmax(x M_h xT) with
M_h = Wq_hT @ Wk_h precomputed HOST-side (a pure weight transformation, like
the lam precompute).  This removes the k-projection entirely and folds the
q-projection into a single xM GEMM: the score path drops from
2*(S*D*D) + S*S*D MACs to S*D*D + S*S*D per half — ~47% less tensor-engine
work in phase A.  Biases fold exactly: row-constant terms are softmax
invariant; the only surviving term is 1·(x · Wk_hT bq_h)ᵀ, a per-j scalar
added during the PSUM→SBUF copy of t = xM.

Everything runs in bf16 on the PE (same 1 col/cycle stream rate as f32r, but
FWL weight loads are fully hidden: measured 216 ns vs 233 ns per matmul) with
fp32 PSUM accumulation; measured end-to-end rel_inf ~7.6e-3.

Per core, a single fused Bass/Tile kernel computes:
  - tT = (x M_h)T j-block by j-block, with the score matmuls fused into the
    same loop (rotating 2-tile window),
  - softmax via ScalarE Exp with accum_out row-sums (scores bounded ~±25,
    no max-subtraction needed in fp32),
  - differential combine P = e1/d1 - lam ⊙ e2/d2 on VectorE,
  - PE-transpose of P, then a fused v-projection + attention-V matmul loop
    that streams Wv (bf16) and writes each 512-column output tile as it
    completes.

Weights are repacked host-side so every DMA lands with long contiguous
per-partition runs.
"""
import ml_dtypes
import numpy as np
from contextlib import ExitStack

import concourse.bass as bass
import concourse.mybir as mybir
import concourse.tile as tile
from concourse.bass_utils import run_bass_kernel_spmd
from concourse.masks import make_identity

F32 = mybir.dt.float32
F32R = mybir.dt.float32r
BF16 = mybir.dt.bfloat16
P = 128
B = 8
S = 512          # sequence length (= d_head for the lambda broadcast)
DM = 4096        # model dim (xM contraction)
DH = 4096        # width of each q/k half (score contraction)
D2 = 2 * DH      # v-projection output width
NQB = S // P     # 4 query blocks
NT = DM // P     # 32 contraction tiles
NXC = 4          # xTb is split into NXC chunk tiles for fine-grained deps
XCT = NT // NXC  # tiles per xTb chunk
HOB = DH // P    # 32 j-blocks per half
NODT = D2 // 512  # 16 output column tiles
WV_CH = 8        # dq-tiles per streamed Wv chunk
NCH = NT // WV_CH
SCALE = float(1.0 / np.sqrt(512.0))

MAX_WAITS = 1  # this walrus build allows one sync-wait per instruction


def _split_sync_waits(nc):
    """Move excess per-instruction sync waits onto preceding no-ops (same
    engine, program order) — semantically identical, keeps walrus happy."""
    for f in nc.m.functions:
        for bb in f.blocks:
            new_insts = []
            for inst in bb.instructions:
                si = inst.sync_info
                if si is not None and si.on_wait and len(si.on_wait) > MAX_WAITS:
                    waits = list(si.on_wait)
                    excess, keep = waits[MAX_WAITS:], waits[:MAX_WAITS]
                    for ci in range(0, len(excess), MAX_WAITS):
                        new_insts.append(mybir.InstNoOp(
                            name=f"{inst.name}-waitsplit{ci}",
                            engine=inst.engine, ins=[], outs=[],
                            sync_info=mybir.SyncInfo(
                                on_wait=excess[ci:ci + MAX_WAITS], on_update=[]),
                            text_hint="waitsplit"))
                    si.on_wait = keep
                new_insts.append(inst)
            bb.instructions = new_insts


def build_nc():
    nc = bass.Bass()
    xTb = nc.declare_dram_parameter("xTb", [P, NT, S], BF16, isOutput=False)
    m = nc.declare_dram_parameter("m", [2, HOB, P, NT, P], BF16, isOutput=False)
    wv = nc.declare_dram_parameter("wv", [NODT, P, NT, 512], BF16, isOutput=False)
    wtld = nc.declare_dram_parameter("wtld", [P, 2 * HOB], F32, isOutput=False)
    vb = nc.declare_dram_parameter("vb", [D2], BF16, isOutput=False)
    lam = nc.declare_dram_parameter("lam", [S], F32R, isOutput=False)
    ones = nc.declare_dram_parameter("ones", [P], F32R, isOutput=False)
    onesb = nc.declare_dram_parameter("onesb", [P], BF16, isOutput=False)
    out = nc.declare_dram_parameter("out", [S, D2], F32, isOutput=True)

    with tile.TileContext(nc) as tc, ExitStack() as top:
        const = top.enter_context(tc.tile_pool(name="const", bufs=1))
        # x^T in bf16, split into NXC chunk tiles so the first matmuls only
        # wait on the first 1 MB chunk, not the whole 4 MB load.
        xcs = []
        for xc in range(NXC):
            xt = const.tile([P, XCT, S], BF16, name=f"xTb_sb{xc}")
            nc.sync.dma_start(xt[:], xTb[:, xc * XCT:(xc + 1) * XCT, :])
            xcs.append(xt)

        def xtile(t):
            return xcs[t // XCT][:, t % XCT, :]

        wtld_sb = const.tile([P, 2 * HOB], F32, name="wtld_sb")
        nc.sync.dma_start(wtld_sb[:], wtld[:])
        lam_sb = const.tile([1, S], F32R, name="lam_sb")
        nc.sync.dma_start(lam_sb[:], lam[None, :])
        ones_row = const.tile([1, P], F32R, name="ones_row")
        nc.sync.dma_start(ones_row[:], ones[None, :])
        onesb_row = const.tile([1, P], BF16, name="onesb_row")
        nc.sync.dma_start(onesb_row[:], onesb[None, :])
        wv0 = const.tile([P, WV_CH, 512], BF16, name="wv0")
        vbt0 = const.tile([1, 512], BF16, name="vbt0")
        ident = const.tile([P, P], F32, name="ident")
        make_identity(nc, ident[:])

        # lam broadcast to all 128 partitions via K=1 matmul
        lam_bc = const.tile([P, S], F32, name="lam_bc")
        with tc.tile_pool(name="ps_misc", bufs=1, space="PSUM") as ps_misc:
            pt = ps_misc.tile([P, 512], F32, name="lam_ps")
            nc.tensor.matmul(pt[:], ones_row[:], lam_sb[:], start=True, stop=True)
            nc.vector.tensor_copy(out=lam_bc[:], in_=pt[:])

        e_sb = const.tile([P, 2, NQB, S], F32, name="e_sb")
        d_sb = const.tile([P, 2, NQB], F32, name="d_sb")
        r_sb = const.tile([P, 2, NQB], F32, name="r_sb")
        P_sb = const.tile([P, NQB, S], F32, name="P_sb")
        PT_sb = const.tile([P, S // P, S], BF16, name="PT_sb")

        # ---- Phase A: fused xM projection + score accumulation ----
        with ExitStack() as phA:
            mw = phA.enter_context(tc.tile_pool(name="mw", bufs=6))
            tsb = phA.enter_context(tc.tile_pool(name="tsb", bufs=6))
            ps_t = phA.enter_context(tc.tile_pool(name="ps_t", bufs=3, space="PSUM"))
            ps_scores = phA.enter_context(tc.tile_pool(name="ps_scores", bufs=5, space="PSUM"))

            for h in range(2):
                sc_tiles = [ps_scores.tile([P, S], F32, name=f"sc_{h}_{qbk}", tag="sc")
                            for qbk in range(NQB)]
                prev_t = None

                def emit_scores(jb, t_sb):
                    for qbk in range(NQB):
                        nc.tensor.matmul(sc_tiles[qbk][:],
                                         t_sb[:, qbk * P:(qbk + 1) * P],
                                         xtile(jb),
                                         start=(jb == 0), stop=(jb == HOB - 1))

                for jb in range(HOB):
                    if h == 1 and jb == HOB - 4:
                        nc.sync.dma_start(wv0[:], wv[0][:, 0:WV_CH, :])
                        nc.sync.dma_start(vbt0[:], vb[None, 0:512])
                    pt_ = ps_t.tile([P, S], F32, name="pt", tag="pp")
                    for cw in range(2):
                        mt = mw.tile([P, NT // 2, P], BF16, name="mt", tag="m")
                        nc.sync.dma_start(mt[:], m[h, jb][:, cw * (NT // 2):(cw + 1) * (NT // 2), :])
                        for tt in range(NT // 2):
                            t = cw * (NT // 2) + tt
                            nc.tensor.matmul(pt_[:], mt[:, tt, :], xtile(t),
                                             start=(t == 0), stop=(t == NT - 1))
                    t_sb = tsb.tile([P, S], BF16, name="t_sb", tag="t")
                    nc.vector.tensor_scalar(t_sb[:], pt_[:],
                                            wtld_sb[:, h * HOB + jb:h * HOB + jb + 1],
                                            None, mybir.AluOpType.add)
                    if prev_t is not None:
                        emit_scores(jb - 1, prev_t)
                    prev_t = t_sb
                emit_scores(HOB - 1, prev_t)
                for qbk in range(NQB):
                    nc.scalar.activation(e_sb[:, h, qbk, :], sc_tiles[qbk][:],
                                         mybir.ActivationFunctionType.Exp,
                                         accum_out=d_sb[:, h, qbk:qbk + 1])

        # ---- Phase B+C: combine + transpose overlapped with v projection/AV ----
        nc.vector.reciprocal(r_sb[:, :, :], d_sb[:, :, :])
        with ExitStack() as phC:
            cmb = phC.enter_context(tc.tile_pool(name="cmb", bufs=2))
            wvp = phC.enter_context(tc.tile_pool(name="wvp", bufs=4))
            vbp = phC.enter_context(tc.tile_pool(name="vbp", bufs=2))
            vbcp = phC.enter_context(tc.tile_pool(name="vbcp", bufs=2))
            vsb = phC.enter_context(tc.tile_pool(name="vsb", bufs=2))
            osb = phC.enter_context(tc.tile_pool(name="osb", bufs=4))
            ps_tr = phC.enter_context(tc.tile_pool(name="ps_tr", bufs=2, space="PSUM"))
            ps_vp = phC.enter_context(tc.tile_pool(name="ps_vp", bufs=4, space="PSUM"))
            ps_av = phC.enter_context(tc.tile_pool(name="ps_av", bufs=2, space="PSUM"))

            def combine_and_transpose():
                for qbk in range(NQB):
                    tmp = cmb.tile([P, S], F32, name="tmp", tag="tmp")
                    nc.vector.tensor_tensor(tmp[:], e_sb[:, 1, qbk, :], lam_bc[:],
                                            mybir.AluOpType.mult)
                    nc.vector.tensor_scalar(tmp[:], tmp[:], r_sb[:, 1, qbk:qbk + 1], None,
                                            mybir.AluOpType.mult)
                    nc.vector.tensor_scalar(P_sb[:, qbk, :], e_sb[:, 0, qbk, :],
                                            r_sb[:, 0, qbk:qbk + 1], None,
                                            mybir.AluOpType.mult)
                    nc.vector.tensor_tensor(P_sb[:, qbk, :], P_sb[:, qbk, :], tmp[:],
                                            mybir.AluOpType.subtract)
                for qbk in range(NQB):
                    for kbk in range(S // P):
                        pt2 = ps_tr.tile([P, P], F32, name="pt2", tag="pt")
                        nc.tensor.transpose(pt2[:], P_sb[:, qbk, kbk * P:(kbk + 1) * P],
                                            ident[:])
                        nc.vector.tensor_copy(out=PT_sb[:, kbk, qbk * P:(qbk + 1) * P],
                                              in_=pt2[:])

            v_tiles = [None] * NODT

            def do_av(odt):
                for qbk in range(NQB):
                    pav = ps_av.tile([P, 512], F32, name="pav", tag="av")
                    for kbk in range(S // P):
                        nc.tensor.matmul(pav[:],
                                         PT_sb[:, kbk, qbk * P:(qbk + 1) * P],
                                         v_tiles[odt][:, kbk, :],
                                         start=(kbk == 0), stop=(kbk == S // P - 1))
                    o_st = osb.tile([P, 512], F32, name="o_st", tag="o")
                    nc.vector.tensor_copy(out=o_st[:], in_=pav[:])
                    nc.sync.dma_start(out[qbk * P:(qbk + 1) * P, odt * 512:(odt + 1) * 512],
                                      o_st[:])

            for odt in range(NODT):
                pv = [ps_vp.tile([P, 512], F32, name=f"pv{sb}", tag="vp")
                      for sb in range(NQB)]
                if odt == 0:
                    vbt = vbt0
                else:
                    vbt = vbp.tile([1, 512], BF16, name="vbt", tag="vb")
                    nc.sync.dma_start(vbt[:], vb[None, odt * 512:(odt + 1) * 512])
                # broadcast this odt's v-bias chunk to all 128 partitions once
                # (one matmul instead of one accumulation-seed matmul per sb)
                pvb = ps_av.tile([P, 512], F32, name="pvb", tag="av")
                nc.tensor.matmul(pvb[:], onesb_row[:], vbt[:], start=True, stop=True)
                vb_bc = vbcp.tile([P, 512], F32, name="vb_bc", tag="vbc")
                nc.vector.tensor_copy(out=vb_bc[:], in_=pvb[:])
                for c in range(NCH):
                    if odt == 0 and c == 0:
                        wvt = wv0
                    else:
                        wvt = wvp.tile([P, WV_CH, 512], BF16, name="wvt", tag="wv")
                        nc.sync.dma_start(wvt[:], wv[odt][:, c * WV_CH:(c + 1) * WV_CH, :])
                    for sb in range(NQB):
                        for tt in range(WV_CH):
                            t = c * WV_CH + tt
                            nc.tensor.matmul(pv[sb][:],
                                             xcs[t // XCT][:, t % XCT, sb * P:(sb + 1) * P],
                                             wvt[:, tt, :],
                                             start=(t == 0), stop=(t == NT - 1))
                v_t = vsb.tile([P, S // P, 512], BF16, name="v_t", tag="v")
                for sb in range(NQB):
                    nc.vector.tensor_tensor(v_t[:, sb, :], pv[sb][:], vb_bc[:],
                                            mybir.AluOpType.add)
                v_tiles[odt] = v_t
                if odt == 0:
                    # PE chews on v-proj(0) while DVE does the combine and the
                    # transposes queue up behind it — hides the softmax tail.
                    combine_and_transpose()
                if odt >= 1:
                    do_av(odt - 1)
                    v_tiles[odt - 1] = None
            do_av(NODT - 1)

    _split_sync_waits(nc)
    return nc


def pack_shared(wq_w, wq_b, wk_w, wk_b, wv_w, wv_b,
                lambda_q1, lambda_k1, lambda_q2, lambda_k2):
    lam = (np.exp(lambda_q1 * lambda_k1) - np.exp(lambda_q2 * lambda_k2)
           + np.float32(0.8)).astype(np.float32)
    # M_h = Wq_h^T @ Wk_h  (scores_h = x M_h x^T); scale folded in.
    ms = []
    wt = []
    for h in range(2):
        wq_h = wq_w[h * DH:(h + 1) * DH]
        wk_h = wk_w[h * DH:(h + 1) * DH]
        M = (wq_h.T @ wk_h) * np.float32(SCALE)
        ms.append(M.reshape(NT, P, HOB, P).transpose(2, 1, 0, 3))
        # surviving bias term: 1·(x · Wk_h^T bq_h)^T, broadcast over q
        w_t = (wk_h.T @ wq_b[h * DH:(h + 1) * DH]) * np.float32(SCALE)
        wt.append(w_t.reshape(HOB, P).T)
    return {
        "m": np.ascontiguousarray(np.stack(ms)).astype(ml_dtypes.bfloat16),
        "wtld": np.ascontiguousarray(np.concatenate(wt, axis=1)),
        "wv": np.ascontiguousarray(wv_w.reshape(NODT, 512, NT, P).transpose(0, 3, 2, 1)).astype(ml_dtypes.bfloat16),
        "vb": np.ascontiguousarray(wv_b).astype(ml_dtypes.bfloat16),
        "lam": lam,
        "ones": np.ones(P, np.float32),
        "onesb": np.ones(P, ml_dtypes.bfloat16),
    }


def make_in_maps(x, wq_w, wq_b, wk_w, wk_b, wv_w, wv_b,
                 lambda_q1, lambda_k1, lambda_q2, lambda_k2):
    shared = pack_shared(wq_w, wq_b, wk_w, wk_b, wv_w, wv_b,
                         lambda_q1, lambda_k1, lambda_q2, lambda_k2)
    maps = []
    for b in range(B):
        xp = np.ascontiguousarray(x[b].T.reshape(NT, P, S).transpose(1, 0, 2))
        maps.append({**shared, "xTb": xp.astype(ml_dtypes.bfloat16)})
    return maps


_NC_CACHE = None


def get_nc():
    global _NC_CACHE
    if _NC_CACHE is None:
        _NC_CACHE = build_nc()
    return _NC_CACHE


def kernel(x, wq_w, wq_b, wk_w, wk_b, wv_w, wv_b,
           lambda_q1, lambda_k1, lambda_q2, lambda_k2):
    args = [np.asarray(a, dtype=np.float32) for a in
            (x, wq_w, wq_b, wk_w, wk_b, wv_w, wv_b,
             lambda_q1, lambda_k1, lambda_q2, lambda_k2)]
    nc = get_nc()
    in_maps = make_in_maps(*args)
    res = run_bass_kernel_spmd(nc, in_maps, list(range(B)))
    return np.stack([res.results[b]["out"] for b in range(B)]).astype(np.float32)


# revision 4
# speedup vs baseline: 1.0882x; 1.0215x over previous
"""Multi-head differential attention (full-width) on 8 Trainium2 NeuronCores.

Contract: kernel(**inputs) takes the FULL inputs of reference.setup_inputs()
and returns the FULL [8, 512, 8192] float32 output.

Strategy: pure data-parallel over batch — one batch element per NeuronCore.

Algebraic optimizations (pure host-side weight transformations, like the
existing lam precompute):
  1. softmax(q_h k_hT) = softmax(x M_h xT) with M_h = Wq_hT @ Wk_h
     precomputed on host — removes the k-projection entirely and folds the
     q-projection into one xM GEMM (~47% less PE work in the score path).
     Biases fold exactly: row-constant terms are softmax-invariant; the
     surviving term 1·(x · Wk_hT bq_h)ᵀ is a per-j scalar added during the
     PSUM→SBUF copy of t = xM.
  2. P @ (x WvT + 1 vbT) = (P @ x) @ WvT + r vbT with r = rowsum(P) —
     computing PX [S, D] first replaces the S²·2D attention·V GEMM with an
     S²·D one (half), and rowsums come free from the combine's accum_out.
     The bias lands in the PSUM-evacuation scalar_tensor_tensor.

Everything runs in bf16 on the PE (same 1 col/cycle stream rate as f32r, but
FWL weight loads fully hide: measured 216 ns vs 233 ns per matmul) with fp32
PSUM accumulation; measured end-to-end rel_inf ~7e-3 (gate 2e-2).

Per core, a single fused Bass/Tile kernel computes:
  - tT = (x M_h)T j-block by j-block, score matmuls fused into the same loop
    (rotating 2-tile window),
  - softmax via ScalarE Exp with accum_out row-sums (scores bounded ~±25,
    no max-subtraction needed in fp32),
  - differential combine P = e1/d1 - lam ⊙ e2/d2 in one scalar_tensor_tensor
    per q-block pair, with accum_out producing rowsum(P),
  - PE-transpose of P, PX = PT-matmuls, then the output GEMM (PX)·WvT
    streaming Wv in bf16, bias added during PSUM evacuation.

Weights are repacked host-side so every DMA lands with long contiguous
per-partition runs.
"""
import ml_dtypes
import numpy as np
from contextlib import ExitStack

import concourse.bass as bass
import concourse.mybir as mybir
import concourse.tile as tile
from concourse.bass_utils import run_bass_kernel_spmd
from concourse.masks import make_identity

F32 = mybir.dt.float32
F32R = mybir.dt.float32r
BF16 = mybir.dt.bfloat16
P = 128
B = 8
S = 512          # sequence length (= d_head for the lambda broadcast)
DM = 4096        # model dim (xM contraction)
DH = 4096        # width of each q/k half (score contraction)
D2 = 2 * DH      # v-projection output width
NQB = S // P     # 4 query blocks
NT = DM // P     # 32 contraction tiles
NXC = 8          # xTb is split into NXC chunk tiles for fine-grained deps
XCT = NT // NXC  # tiles per xTb chunk
HOB = DH // P    # 32 j-blocks per half
NODT = D2 // 512  # 16 output column tiles
WV_CH = 8        # i-tiles per streamed Wv chunk
NCH = NT // WV_CH
SCALE = float(1.0 / np.sqrt(512.0))

MAX_WAITS = 1  # this walrus build allows one sync-wait per instruction


def _split_sync_waits(nc):
    """Move excess per-instruction sync waits onto preceding no-ops (same
    engine, program order) — semantically identical, keeps walrus happy."""
    for f in nc.m.functions:
        for bb in f.blocks:
            new_insts = []
            for inst in bb.instructions:
                si = inst.sync_info
                if si is not None and si.on_wait and len(si.on_wait) > MAX_WAITS:
                    waits = list(si.on_wait)
                    excess, keep = waits[MAX_WAITS:], waits[:MAX_WAITS]
                    for ci in range(0, len(excess), MAX_WAITS):
                        new_insts.append(mybir.InstNoOp(
                            name=f"{inst.name}-waitsplit{ci}",
                            engine=inst.engine, ins=[], outs=[],
                            sync_info=mybir.SyncInfo(
                                on_wait=excess[ci:ci + MAX_WAITS], on_update=[]),
                            text_hint="waitsplit"))
                    si.on_wait = keep
                new_insts.append(inst)
            bb.instructions = new_insts


def build_nc():
    nc = bass.Bass()
    xTb = nc.declare_dram_parameter("xTb", [P, NT, S], BF16, isOutput=False)
    xn = nc.declare_dram_parameter("xn", [P, NQB, DM], BF16, isOutput=False)
    m = nc.declare_dram_parameter("m", [2, HOB, P, NT, P], BF16, isOutput=False)
    wv = nc.declare_dram_parameter("wv", [NODT, P, NT, 512], BF16, isOutput=False)
    wtld = nc.declare_dram_parameter("wtld", [P, 2 * HOB], F32, isOutput=False)
    vb = nc.declare_dram_parameter("vb", [D2], BF16, isOutput=False)
    lam = nc.declare_dram_parameter("lam", [S], F32R, isOutput=False)
    ones = nc.declare_dram_parameter("ones", [P], F32R, isOutput=False)
    onesb = nc.declare_dram_parameter("onesb", [P], BF16, isOutput=False)
    out = nc.declare_dram_parameter("out", [S, D2], F32, isOutput=True)

    with tile.TileContext(nc) as tc, ExitStack() as top:
        const = top.enter_context(tc.tile_pool(name="const", bufs=1))
        # First x chunk + first M chunk first in program order: these gate the
        # very first matmul, everything else streams in behind them.
        xcs = [const.tile([P, XCT, S], BF16, name=f"xTb_sb{xc}")
               for xc in range(NXC)]
        nc.sync.dma_start(xcs[0][:], xTb[:, 0:XCT, :])
        mt0 = const.tile([P, NT // 2, P], BF16, name="mt0")
        nc.sync.dma_start(mt0[:], m[0, 0][:, 0:NT // 2, :])
        wtld_sb = const.tile([P, 2 * HOB], F32, name="wtld_sb")
        nc.sync.dma_start(wtld_sb[:], wtld[:])
        for xc in range(1, NXC):
            nc.sync.dma_start(xcs[xc][:], xTb[:, xc * XCT:(xc + 1) * XCT, :])

        def xtile(t):
            return xcs[t // XCT][:, t % XCT, :]

        lam_sb = const.tile([1, S], F32R, name="lam_sb")
        nc.sync.dma_start(lam_sb[:], lam[None, :])
        ones_row = const.tile([1, P], F32R, name="ones_row")
        nc.sync.dma_start(ones_row[:], ones[None, :])
        onesb_row = const.tile([1, P], BF16, name="onesb_row")
        nc.sync.dma_start(onesb_row[:], onesb[None, :])
        wv0 = const.tile([P, WV_CH, 512], BF16, name="wv0")
        vbt0 = const.tile([1, 512], BF16, name="vbt0")
        xn_sb = const.tile([P, NQB, DM], BF16, name="xn_sb")
        ident = const.tile([P, P], F32, name="ident")
        make_identity(nc, ident[:])
        lam_bc = const.tile([P, S], F32, name="lam_bc")

        e_sb = const.tile([P, 2, NQB, S], F32, name="e_sb")
        d_sb = const.tile([P, 2, NQB], F32, name="d_sb")
        r_sb = const.tile([P, 2, NQB], F32, name="r_sb")
        rp_sb = const.tile([P, NQB], F32, name="rp_sb")
        P_sb = const.tile([P, NQB, S], F32, name="P_sb")
        PT_sb = const.tile([P, S // P, S], BF16, name="PT_sb")
        PXT_sb = const.tile([P, NT, S], BF16, name="PXT_sb")

        # ---- Phase A: fused xM projection + score accumulation ----
        with ExitStack() as phA:
            mw = phA.enter_context(tc.tile_pool(name="mw", bufs=6))
            tsb = phA.enter_context(tc.tile_pool(name="tsb", bufs=6))
            ps_t = phA.enter_context(tc.tile_pool(name="ps_t", bufs=2, space="PSUM"))
            ps_misc = phA.enter_context(tc.tile_pool(name="ps_misc", bufs=1, space="PSUM"))
            ps_scores = phA.enter_context(tc.tile_pool(name="ps_scores", bufs=5, space="PSUM"))

            for h in range(2):
                sc_tiles = [ps_scores.tile([P, S], F32, name=f"sc_{h}_{qbk}", tag="sc")
                            for qbk in range(NQB)]
                prev_t = None

                def emit_scores(jb, t_sb):
                    for qbk in range(NQB):
                        nc.tensor.matmul(sc_tiles[qbk][:],
                                         t_sb[:, qbk * P:(qbk + 1) * P],
                                         xtile(jb),
                                         start=(jb == 0), stop=(jb == HOB - 1))

                for jb in range(HOB):
                    if h == 1:
                        if jb == 2:
                            # lam broadcast to all 128 partitions via K=1
                            # matmul (emitted here, off the PE queue head)
                            plam = ps_misc.tile([P, 512], F32, name="lam_ps")
                            nc.tensor.matmul(plam[:], ones_row[:], lam_sb[:],
                                             start=True, stop=True)
                            nc.vector.tensor_copy(out=lam_bc[:], in_=plam[:])
                        elif jb == 8:
                            nc.sync.dma_start(xn_sb[:], xn[:])
                        elif jb == 16:
                            nc.sync.dma_start(wv0[:], wv[0][:, 0:WV_CH, :])
                            nc.sync.dma_start(vbt0[:], vb[None, 0:512])
                    pt_ = ps_t.tile([P, S], F32, name="pt", tag="pp")
                    for cw in range(2):
                        if h == 0 and jb == 0 and cw == 0:
                            mt = mt0
                        else:
                            mt = mw.tile([P, NT // 2, P], BF16, name="mt", tag="m")
                            nc.sync.dma_start(mt[:], m[h, jb][:, cw * (NT // 2):(cw + 1) * (NT // 2), :])
                        for tt in range(NT // 2):
                            t = cw * (NT // 2) + tt
                            nc.tensor.matmul(pt_[:], mt[:, tt, :], xtile(t),
                                             start=(t == 0), stop=(t == NT - 1))
                    t_sb = tsb.tile([P, S], BF16, name="t_sb", tag="t")
                    nc.vector.tensor_scalar(t_sb[:], pt_[:],
                                            wtld_sb[:, h * HOB + jb:h * HOB + jb + 1],
                                            None, mybir.AluOpType.add)
                    if prev_t is not None:
                        emit_scores(jb - 1, prev_t)
                    prev_t = t_sb
                emit_scores(HOB - 1, prev_t)
                for qbk in range(NQB):
                    nc.scalar.activation(e_sb[:, h, qbk, :], sc_tiles[qbk][:],
                                         mybir.ActivationFunctionType.Exp,
                                         accum_out=d_sb[:, h, qbk:qbk + 1])

        # ---- Phase B+C: combine + transpose + PX, then output GEMM ----
        nc.vector.reciprocal(r_sb[:, :, :], d_sb[:, :, :])
        with ExitStack() as phC:
            cmb = phC.enter_context(tc.tile_pool(name="cmb", bufs=2))
            wvp = phC.enter_context(tc.tile_pool(name="wvp", bufs=4))
            vbp = phC.enter_context(tc.tile_pool(name="vbp", bufs=2))
            vbcp = phC.enter_context(tc.tile_pool(name="vbcp", bufs=2))
            osb = phC.enter_context(tc.tile_pool(name="osb", bufs=4))
            ps_tmp = phC.enter_context(tc.tile_pool(name="ps_tmp", bufs=2, space="PSUM"))
            ps_fin = phC.enter_context(tc.tile_pool(name="ps_fin", bufs=6, space="PSUM"))

            # combine: P = e1/d1 - lam ⊙ e2/d2, rowsum(P) via accum_out;
            # then PE-transpose P into PT, then PX^T = x^T P^T via PE.
            for qbk in range(NQB):
                tmp = cmb.tile([P, S], F32, name="tmp", tag="tmp")
                nc.vector.scalar_tensor_tensor(
                    tmp[:], e_sb[:, 1, qbk, :], r_sb[:, 1, qbk:qbk + 1], lam_bc[:],
                    mybir.AluOpType.mult, mybir.AluOpType.mult)
                nc.vector.scalar_tensor_tensor(
                    P_sb[:, qbk, :], e_sb[:, 0, qbk, :], r_sb[:, 0, qbk:qbk + 1],
                    tmp[:], mybir.AluOpType.mult, mybir.AluOpType.subtract,
                    accum_out=rp_sb[:, qbk:qbk + 1])
                for kbk in range(S // P):
                    pt2 = ps_tmp.tile([P, P], F32, name="pt2", tag="pt")
                    nc.tensor.transpose(pt2[:], P_sb[:, qbk, kbk * P:(kbk + 1) * P],
                                        ident[:])
                    nc.vector.tensor_copy(out=PT_sb[:, kbk, qbk * P:(qbk + 1) * P],
                                          in_=pt2[:])
            for it in range(NT):
                ppx = ps_tmp.tile([P, S], F32, name="ppx", tag="pt")
                for kbk in range(NQB):
                    nc.tensor.matmul(ppx[:], xn_sb[:, kbk, it * P:(it + 1) * P],
                                     PT_sb[:, kbk, :],
                                     start=(kbk == 0), stop=(kbk == NQB - 1))
                nc.vector.tensor_copy(out=PXT_sb[:, it, :], in_=ppx[:])

            # output GEMM: out[:, odt] = (PX @ WvT)[:, odt] + rowsum(P)·vb
            for odt in range(NODT):
                pv = [ps_fin.tile([P, 512], F32, name=f"pv{qbk}", tag="vp")
                      for qbk in range(NQB)]
                if odt == 0:
                    vbt = vbt0
                else:
                    vbt = vbp.tile([1, 512], BF16, name="vbt", tag="vb")
                    nc.sync.dma_start(vbt[:], vb[None, odt * 512:(odt + 1) * 512])
                # broadcast this odt's v-bias chunk to all 128 partitions
                pvb = ps_tmp.tile([P, 512], F32, name="pvb", tag="pt")
                nc.tensor.matmul(pvb[:], onesb_row[:], vbt[:], start=True, stop=True)
                vb_bc = vbcp.tile([P, 512], F32, name="vb_bc", tag="vbc")
                nc.vector.tensor_copy(out=vb_bc[:], in_=pvb[:])
                for c in range(NCH):
                    if odt == 0 and c == 0:
                        wvt = wv0
                    else:
                        wvt = wvp.tile([P, WV_CH, 512], BF16, name="wvt", tag="wv")
                        nc.sync.dma_start(wvt[:], wv[odt][:, c * WV_CH:(c + 1) * WV_CH, :])
                    for qbk in range(NQB):
                        for tt in range(WV_CH):
                            t = c * WV_CH + tt
                            nc.tensor.matmul(pv[qbk][:],
                                             PXT_sb[:, t, qbk * P:(qbk + 1) * P],
                                             wvt[:, tt, :],
                                             start=(t == 0), stop=(t == NT - 1))
                for qbk in range(NQB):
                    o_st = osb.tile([P, 512], F32, name="o_st", tag="o")
                    nc.vector.scalar_tensor_tensor(
                        o_st[:], vb_bc[:], rp_sb[:, qbk:qbk + 1], pv[qbk][:],
                        mybir.AluOpType.mult, mybir.AluOpType.add)
                    nc.sync.dma_start(out[qbk * P:(qbk + 1) * P, odt * 512:(odt + 1) * 512],
                                      o_st[:])

    _split_sync_waits(nc)
    return nc


def pack_shared(wq_w, wq_b, wk_w, wk_b, wv_w, wv_b,
                lambda_q1, lambda_k1, lambda_q2, lambda_k2):
    lam = (np.exp(lambda_q1 * lambda_k1) - np.exp(lambda_q2 * lambda_k2)
           + np.float32(0.8)).astype(np.float32)
    # M_h = Wq_h^T @ Wk_h  (scores_h = x M_h x^T); scale folded in.
    ms = []
    wt = []
    for h in range(2):
        wq_h = wq_w[h * DH:(h + 1) * DH]
        wk_h = wk_w[h * DH:(h + 1) * DH]
        M = (wq_h.T @ wk_h) * np.float32(SCALE)
        ms.append(M.reshape(NT, P, HOB, P).transpose(2, 1, 0, 3))
        # surviving bias term: 1·(x · Wk_h^T bq_h)^T, broadcast over q
        w_t = (wk_h.T @ wq_b[h * DH:(h + 1) * DH]) * np.float32(SCALE)
        wt.append(w_t.reshape(HOB, P).T)
    return {
        "m": np.ascontiguousarray(np.stack(ms)).astype(ml_dtypes.bfloat16),
        "wtld": np.ascontiguousarray(np.concatenate(wt, axis=1)),
        "wv": np.ascontiguousarray(wv_w.reshape(NODT, 512, NT, P).transpose(0, 3, 2, 1)).astype(ml_dtypes.bfloat16),
        "vb": np.ascontiguousarray(wv_b).astype(ml_dtypes.bfloat16),
        "lam": lam,
        "ones": np.ones(P, np.float32),
        "onesb": np.ones(P, ml_dtypes.bfloat16),
    }


def make_in_maps(x, wq_w, wq_b, wk_w, wk_b, wv_w, wv_b,
                 lambda_q1, lambda_k1, lambda_q2, lambda_k2):
    shared = pack_shared(wq_w, wq_b, wk_w, wk_b, wv_w, wv_b,
                         lambda_q1, lambda_k1, lambda_q2, lambda_k2)
    maps = []
    for b in range(B):
        xp = np.ascontiguousarray(x[b].T.reshape(NT, P, S).transpose(1, 0, 2))
        xnp = np.ascontiguousarray(x[b].reshape(NQB, P, DM).transpose(1, 0, 2))
        maps.append({**shared,
                     "xTb": xp.astype(ml_dtypes.bfloat16),
                     "xn": xnp.astype(ml_dtypes.bfloat16)})
    return maps


_NC_CACHE = None


def get_nc():
    global _NC_CACHE
    if _NC_CACHE is None:
        _NC_CACHE = build_nc()
    return _NC_CACHE


def kernel(x, wq_w, wq_b, wk_w, wk_b, wv_w, wv_b,
           lambda_q1, lambda_k1, lambda_q2, lambda_k2):
    args = [np.asarray(a, dtype=np.float32) for a in
            (x, wq_w, wq_b, wk_w, wk_b, wv_w, wv_b,
             lambda_q1, lambda_k1, lambda_q2, lambda_k2)]
    nc = get_nc()
    in_maps = make_in_maps(*args)
    res = run_bass_kernel_spmd(nc, in_maps, list(range(B)))
    return np.stack([res.results[b]["out"] for b in range(B)]).astype(np.float32)
